# revision 27
# baseline (speedup 1.0000x reference)
"""Trainium2 Bass kernel for nn_ExpandEvecs.

Computes, for evecs [B=4, C=1, N=1024, K=16]:
    cube[b,l] = V[:, :l+1] @ V[:, :l+1]^T   (Gram expansion per level)
    -> [B, K, N, N] fp32 (cumsum of per-eigvec outer products over l).

Sharding: 8 cores = 4 batches x 2 row-halves; core c (b=c//2, h=c%2)
produces all 16 levels for its 512-row half. No communication.

Performance model (per core, 8.4M output elements; all rates HW-measured):
  - The PE streams one 512-column matmul per 427 ns (1.2 GHz sustained;
    the 2.4 GHz p-state needs 3 us of gapless execution, unreachable
    when PSUM drain paces the PE) -> 128 matmuls = 54.7 us. This is the
    kernel's floor: the PE is the only engine that can produce outer
    products at rate (GpSimd tensor ops measured 2.1-15 us per 131K
    elems, DVE fused STT 9.4 us -> offload designs all lose).
  - PSUM evacuation: only ACT (1.2 GHz) and DVE (0.96 GHz) have PSUM
    ports. Whole [128, 2048] ops (one 4-bank PSUM tile) alternate
    between them ~53:47 -> ~35 us in parallel, under the PE floor.
  - int8 output (8.4 MB -> ~23.5 us at the ~358 GB/s per-core HBM
    limit) keeps DMA far off the critical path; fp32 would be 94 us.

Precision (gate 2e-2; simulated end-to-end 4.5e-3):
  - fp8 split matmuls: V = A + B, A = e4m3(V), B = e4m3(V-A);
    V V^T ~= A A^T + A B^T + B A^T (dropped B B^T ~2^-8). 4 stacked
    rows per eigvec k (lhs A,B,A,0 / rhs A,A,B,0) so DoubleRow pairs
    never straddle a k boundary.
  - int8 scale per (level, partition) from the Cauchy-Schwarz bound
    max over the partition's 4 interleaved rows of
    ||v_i||_l * max_j ||v_j||_l (host-computed, 2% margin), applied
    during evacuation (ACT activation scale= / DVE tensor_scalar,
    which round to nearest). Host dequantizes during the unshard.
  - Row-pair interleave: partition p holds DRAM rows 4p..4p+3, giving
    4 KiB contiguous int8 store runs per partition.
"""

import numpy as np
import ml_dtypes

import concourse.mybir as mybir
from concourse import bacc, bass
from concourse.tile import TileContext
from concourse.bass_utils import run_bass_kernel_spmd

B, C, N, K = 4, 1, 1024, 16
NCORES = 8
HALF = N // 2          # rows per core
KI = 2 * K             # DoubleRow pair-partitions at the deepest level

F32 = mybir.dt.float32
FP8 = mybir.dt.float8e4
I8 = mybir.dt.int8
FP8_NP = ml_dtypes.float8_e4m3

_nc_cache = None


def _build():
    nc = bacc.Bacc(None, target_bir_lowering=False)
    t_d = nc.declare_dram_parameter("t", [KI, 2, N], FP8, isOutput=False)
    tl_d = nc.declare_dram_parameter("tl", [KI, 2, HALF], FP8, isOutput=False)
    sc_d = nc.declare_dram_parameter("sc", [128, K * 4], F32, isOutput=False)
    out_d = nc.declare_dram_parameter("out", [K, HALF, N], I8, isOutput=True)

    DR = mybir.MatmulPerfMode.DoubleRow
    COPY = mybir.ActivationFunctionType.Copy
    MUL = mybir.AluOpType.mult
    acc = [0]

    with TileContext(nc) as tc:
        with (
            tc.tile_pool(name="vpool", bufs=1) as vpool,
            tc.tile_pool(name="stage", bufs=3) as stage,
            tc.tile_pool(name="psum", bufs=4, space=bass.MemorySpace.PSUM) as psum,
        ):
            t = vpool.tile([KI, 2, N], FP8)
            tl = vpool.tile([KI, 2, HALF], FP8)
            sc = vpool.tile([128, K * 4], F32)
            t0 = vpool.tile([2, 2, N], FP8)
            tl0 = vpool.tile([2, 2, HALF], FP8)
            # tiny level-0 slices land first and unblock the PE earlier
            # than the full stacks; two HWDGE rings
            nc.sync.dma_start(out=tl0[:], in_=tl_d[:2])
            nc.scalar.dma_start(out=t0[:], in_=t_d[:2])
            nc.sync.dma_start(out=sc[:], in_=sc_d[:])
            nc.sync.dma_start(out=tl[:], in_=tl_d[:])
            nc.scalar.dma_start(out=t[:], in_=t_d[:])

            tlv = tl.rearrange("k o (m r) -> k o m r", m=128, r=4)
            tlv0 = tl0.rearrange("k o (m r) -> k o m r", m=128, r=4)

            # levels 0-13 store in pairs (halves the DMA-completion count
            # the epilogue polls); 14 alone; 15 drains per-r
            st2 = None
            for lvl in range(K):
                ki = 2 * (lvl + 1)
                lhs_all, rhs_all = (tlv0, t0) if lvl < 1 else (tlv, t)
                tail = lvl == K - 1
                if lvl < 14:
                    if lvl % 2 == 0:
                        st2 = stage.tile([128, 2, 4, N], I8, tag="st2")
                    st = st2[:, lvl % 2]
                else:
                    st = stage.tile([128, 4, N], I8, tag="st")
                for r in range(4):
                    ps = psum.tile([128, N], F32, tag="ps")  # 2 banks
                    for j in range(2):
                        nc.tensor.matmul(
                            ps[:, j * 512:(j + 1) * 512],
                            lhsT=lhs_all[:ki, :, :, r],
                            rhs=rhs_all[:ki, :, j * 512:(j + 1) * 512],
                            start=True, stop=True, perf_mode=DR,
                        )
                    s_ap = sc[:, 4 * lvl + r:4 * lvl + r + 1]
                    if tail:
                        # pipeline drain: both engines in parallel on the
                        # tile's two banks, store each r-slice immediately
                        nc.scalar.activation(st[:, r, :512], ps[:, :512],
                                             COPY, scale=s_ap)
                        nc.vector.tensor_scalar(st[:, r, 512:],
                                                ps[:, 512:], s_ap, None, MUL)
                        nc.sync.dma_start(
                            out=out_d[lvl].rearrange(
                                "(p r) f -> p r f", p=128)[:, r, :],
                            in_=st[:, r, :])
                        continue
                    # [128, 1024] scale+cast evacuation, alternating
                    # ACT:DVE ~ 8:7 (their measured op-rate ratio)
                    acc[0] += 8
                    if acc[0] >= 15:
                        acc[0] -= 15
                        nc.scalar.activation(st[:, r, :], ps[:],
                                             COPY, scale=s_ap)
                    else:
                        nc.vector.tensor_scalar(st[:, r, :],
                                                ps[:], s_ap, None, MUL)
                if lvl < 14:
                    if lvl % 2 == 1:
                        nc.sync.dma_start(
                            out=out_d[lvl - 1:lvl + 1].rearrange(
                                "l (p r) f -> p l r f", p=128),
                            in_=st2[:, :, :, :],
                        )
                elif not tail:
                    nc.sync.dma_start(
                        out=out_d[lvl].rearrange("(p r) f -> p r f", p=128),
                        in_=st[:, :, :],
                    )

    nc.compile()
    return nc


def _get_nc():
    global _nc_cache
    if _nc_cache is None:
        _nc_cache = _build()
    return _nc_cache


def _prepare_in_maps(evecs: np.ndarray):
    in_maps = []
    bounds = []
    for c in range(NCORES):
        b, h = divmod(c, 2)
        vt = np.ascontiguousarray(evecs[b, 0].T, dtype=np.float32)  # [K, N]
        a32 = vt.astype(FP8_NP).astype(np.float32)
        b32 = (vt - a32).astype(FP8_NP).astype(np.float32)
        sl = slice(h * HALF, (h + 1) * HALF)

        rhs = np.zeros((4 * K, N), dtype=np.float32)
        rhs[0::4] = a32
        rhs[1::4] = a32
        rhs[2::4] = b32
        lhs = np.zeros((4 * K, HALF), dtype=np.float32)
        lhs[0::4] = a32[:, sl]
        lhs[1::4] = b32[:, sl]
        lhs[2::4] = a32[:, sl]
        t = rhs.reshape(KI, 2, N).astype(FP8_NP)
        tl = lhs.reshape(KI, 2, HALF).astype(FP8_NP)

        # Cauchy-Schwarz bound -> per-(level, row) int8 scale, 2% margin
        cn = np.sqrt(np.cumsum(vt * vt, axis=0))          # [K, N]
        maxn = cn.max(axis=1)                             # [K]
        bound = cn[:, sl] * maxn[:, None] * 1.02          # [K, HALF]
        s = (127.0 / bound).astype(np.float32)
        # sc[p, 4*l + r] = s[l, 4p + r]
        sc = np.ascontiguousarray(
            s.reshape(K, 128, 4).transpose(1, 0, 2).reshape(128, K * 4)
        )
        in_maps.append({"t": t, "tl": tl, "sc": sc})
        bounds.append(bound)                              # [K, HALF]
    return in_maps, bounds


def _assemble(results, bounds) -> np.ndarray:
    out = np.empty((B, K, N, N), dtype=np.float32)
    for c in range(NCORES):
        b, h = divmod(c, 2)
        q = results[c]["out"].astype(np.float32)          # [K, HALF, N]
        q *= (bounds[c] / 127.0)[:, :, None]
        out[b, :, h * HALF:(h + 1) * HALF, :] = q
    return out.reshape(B, K * C, N, N)


def kernel(evecs) -> np.ndarray:
    evecs = np.asarray(evecs, dtype=np.float32)
    assert evecs.shape == (B, C, N, K), evecs.shape
    nc = _get_nc()
    in_maps, bounds = _prepare_in_maps(evecs)
    last_err = None
    for _attempt in range(3):
        try:
            r = run_bass_kernel_spmd(nc, in_maps, list(range(NCORES)))
            return _assemble(r.results, bounds)
        except Exception as e:  # transient NRT/device hiccups: retry
            last_err = e
    raise last_err


# revision 29
# speedup vs baseline: 1.0032x; 1.0032x over previous
"""Trainium2 Bass kernel for nn_ExpandEvecs.

Computes, for evecs [B=4, C=1, N=1024, K=16]:
    cube[b,l] = V[:, :l+1] @ V[:, :l+1]^T   (Gram expansion per level)
    -> [B, K, N, N] fp32 (cumsum of per-eigvec outer products over l).

Sharding: 8 cores = 4 batches x 2 row-halves; core c (b=c//2, h=c%2)
produces all 16 levels for its 512-row half. No communication.

Performance model (per core, 8.4M output elements; all rates HW-measured):
  - The PE streams one 512-column matmul per 427 ns (1.2 GHz sustained;
    the 2.4 GHz p-state needs 3 us of gapless execution, unreachable
    when PSUM drain paces the PE) -> 128 matmuls = 54.7 us. This is the
    kernel's floor: the PE is the only engine that can produce outer
    products at rate (GpSimd tensor ops measured 2.1-15 us per 131K
    elems, DVE fused STT 9.4 us -> offload designs all lose).
  - PSUM evacuation: only ACT (1.2 GHz) and DVE (0.96 GHz) have PSUM
    ports. Whole [128, 2048] ops (one 4-bank PSUM tile) alternate
    between them ~53:47 -> ~35 us in parallel, under the PE floor.
  - int8 output (8.4 MB -> ~23.5 us at the ~358 GB/s per-core HBM
    limit) keeps DMA far off the critical path; fp32 would be 94 us.

Precision (gate 2e-2; simulated end-to-end 4.5e-3):
  - fp8 split matmuls: V = A + B, A = e4m3(V), B = e4m3(V-A);
    V V^T ~= A A^T + A B^T + B A^T (dropped B B^T ~2^-8). 4 stacked
    rows per eigvec k (lhs A,B,A,0 / rhs A,A,B,0) so DoubleRow pairs
    never straddle a k boundary.
  - int8 scale per (level, partition) from the Cauchy-Schwarz bound
    max over the partition's 4 interleaved rows of
    ||v_i||_l * max_j ||v_j||_l (host-computed, 2% margin), applied
    during evacuation (ACT activation scale= / DVE tensor_scalar,
    which round to nearest). Host dequantizes during the unshard.
  - Row-pair interleave: partition p holds DRAM rows 4p..4p+3, giving
    4 KiB contiguous int8 store runs per partition.
"""

import numpy as np
import ml_dtypes

import concourse.mybir as mybir
from concourse import bacc, bass
from concourse.tile import TileContext
from concourse.bass_utils import run_bass_kernel_spmd

B, C, N, K = 4, 1, 1024, 16
NCORES = 8
HALF = N // 2          # rows per core
KI = 2 * K             # DoubleRow pair-partitions at the deepest level

F32 = mybir.dt.float32
FP8 = mybir.dt.float8e4
I8 = mybir.dt.int8
FP8_NP = ml_dtypes.float8_e4m3

_nc_cache = None


def _build():
    nc = bacc.Bacc(None, target_bir_lowering=False)
    t_d = nc.declare_dram_parameter("t", [KI, 2, N], FP8, isOutput=False)
    tl_d = nc.declare_dram_parameter("tl", [KI, 2, HALF], FP8, isOutput=False)
    sc_d = nc.declare_dram_parameter("sc", [128, K * 4], F32, isOutput=False)
    out_d = nc.declare_dram_parameter("out", [K, HALF, N], I8, isOutput=True)

    DR = mybir.MatmulPerfMode.DoubleRow
    COPY = mybir.ActivationFunctionType.Copy
    MUL = mybir.AluOpType.mult
    acc = [0]

    with TileContext(nc) as tc:
        with (
            tc.tile_pool(name="vpool", bufs=1) as vpool,
            tc.tile_pool(name="stage", bufs=3) as stage,
            tc.tile_pool(name="psum", bufs=4, space=bass.MemorySpace.PSUM) as psum,
        ):
            t = vpool.tile([KI, 2, N], FP8)
            tl = vpool.tile([KI, 2, HALF], FP8)
            sc = vpool.tile([128, K * 4], F32)
            t0 = vpool.tile([2, 2, N], FP8)
            tl0 = vpool.tile([2, 2, HALF], FP8)
            # tiny level-0 slices land first and unblock the PE earlier
            # than the full stacks; two HWDGE rings
            nc.sync.dma_start(out=tl0[:], in_=tl_d[:2])
            nc.scalar.dma_start(out=t0[:], in_=t_d[:2])
            nc.sync.dma_start(out=sc[:], in_=sc_d[:])
            nc.sync.dma_start(out=tl[:], in_=tl_d[:])
            nc.scalar.dma_start(out=t[:], in_=t_d[:])

            tlv = tl.rearrange("k o (m r) -> k o m r", m=128, r=4)
            tlv0 = tl0.rearrange("k o (m r) -> k o m r", m=128, r=4)

            for lvl in range(K):
                ki = 2 * (lvl + 1)
                lhs_all, rhs_all = (tlv0, t0) if lvl < 1 else (tlv, t)
                tail = lvl == K - 1
                st = stage.tile([128, 4, N], I8, tag="st")
                for r in range(4):
                    ps = psum.tile([128, N], F32, tag="ps")  # 2 banks
                    for j in range(2):
                        nc.tensor.matmul(
                            ps[:, j * 512:(j + 1) * 512],
                            lhsT=lhs_all[:ki, :, :, r],
                            rhs=rhs_all[:ki, :, j * 512:(j + 1) * 512],
                            start=True, stop=True, perf_mode=DR,
                        )
                    s_ap = sc[:, 4 * lvl + r:4 * lvl + r + 1]
                    if tail:
                        # pipeline drain: both engines in parallel on the
                        # tile's two banks, store each r-slice immediately
                        nc.scalar.activation(st[:, r, :512], ps[:, :512],
                                             COPY, scale=s_ap)
                        nc.vector.tensor_scalar(st[:, r, 512:],
                                                ps[:, 512:], s_ap, None, MUL)
                        nc.sync.dma_start(
                            out=out_d[lvl].rearrange(
                                "(p r) f -> p r f", p=128)[:, r, :],
                            in_=st[:, r, :])
                        continue
                    # [128, 1024] scale+cast evacuation, alternating
                    # ACT:DVE ~ 8:7 (their measured op-rate ratio)
                    acc[0] += 8
                    if acc[0] >= 15:
                        acc[0] -= 15
                        nc.scalar.activation(st[:, r, :], ps[:],
                                             COPY, scale=s_ap)
                    else:
                        nc.vector.tensor_scalar(st[:, r, :],
                                                ps[:], s_ap, None, MUL)
                if not tail:
                    nc.sync.dma_start(
                        out=out_d[lvl].rearrange("(p r) f -> p r f", p=128),
                        in_=st[:, :, :],
                    )

    nc.compile()
    return nc


def _get_nc():
    global _nc_cache
    if _nc_cache is None:
        _nc_cache = _build()
    return _nc_cache


def _prepare_in_maps(evecs: np.ndarray):
    in_maps = []
    bounds = []
    for c in range(NCORES):
        b, h = divmod(c, 2)
        vt = np.ascontiguousarray(evecs[b, 0].T, dtype=np.float32)  # [K, N]
        a32 = vt.astype(FP8_NP).astype(np.float32)
        b32 = (vt - a32).astype(FP8_NP).astype(np.float32)
        sl = slice(h * HALF, (h + 1) * HALF)

        rhs = np.zeros((4 * K, N), dtype=np.float32)
        rhs[0::4] = a32
        rhs[1::4] = a32
        rhs[2::4] = b32
        lhs = np.zeros((4 * K, HALF), dtype=np.float32)
        lhs[0::4] = a32[:, sl]
        lhs[1::4] = b32[:, sl]
        lhs[2::4] = a32[:, sl]
        t = rhs.reshape(KI, 2, N).astype(FP8_NP)
        tl = lhs.reshape(KI, 2, HALF).astype(FP8_NP)

        # Cauchy-Schwarz bound -> per-(level, row) int8 scale, 2% margin
        cn = np.sqrt(np.cumsum(vt * vt, axis=0))          # [K, N]
        maxn = cn.max(axis=1)                             # [K]
        bound = cn[:, sl] * maxn[:, None] * 1.02          # [K, HALF]
        s = (127.0 / bound).astype(np.float32)
        # sc[p, 4*l + r] = s[l, 4p + r]
        sc = np.ascontiguousarray(
            s.reshape(K, 128, 4).transpose(1, 0, 2).reshape(128, K * 4)
        )
        in_maps.append({"t": t, "tl": tl, "sc": sc})
        bounds.append(bound)                              # [K, HALF]
    return in_maps, bounds


def _assemble(results, bounds) -> np.ndarray:
    out = np.empty((B, K, N, N), dtype=np.float32)
    for c in range(NCORES):
        b, h = divmod(c, 2)
        q = results[c]["out"].astype(np.float32)          # [K, HALF, N]
        q *= (bounds[c] / 127.0)[:, :, None]
        out[b, :, h * HALF:(h + 1) * HALF, :] = q
    return out.reshape(B, K * C, N, N)


def kernel(evecs) -> np.ndarray:
    evecs = np.asarray(evecs, dtype=np.float32)
    assert evecs.shape == (B, C, N, K), evecs.shape
    nc = _get_nc()
    in_maps, bounds = _prepare_in_maps(evecs)
    last_err = None
    for _attempt in range(3):
        try:
            r = run_bass_kernel_spmd(nc, in_maps, list(range(NCORES)))
            return _assemble(r.results, bounds)
        except Exception as e:  # transient NRT/device hiccups: retry
            last_err = e
    raise last_err
